# revision 14
# baseline (speedup 1.0000x reference)
"""Trainium2 kernel for nn_Attention_26774826124067.

Math: the reference module's score einsum sums heads out ('bqhe,bkhe->bqk')
and its value einsum sums the key axis out of the probabilities
('bqk,bqhe->bqhe').  Softmax rows sum to 1, so z == V exactly and the whole
module collapses to

    out[b,q,:] = x[b,q,:] @ M + b,   M = sum_h W_V[h] @ W_O[h]  (D x D),
    b = b_O + b_V_flat @ Wo2

independent of W_Q/W_K/b_Q/b_K.  M and b are folded on the host (17 GFLOP,
like the baseline's b_V fold); the device does the row-sharded GEMM
    outT_i = (x[rows_i] @ M + b)^T          rows_i = 1024 rows per core
with no collectives.  Per core that is 4.3e9 bf16 MACs (~109us at the PE's
78.6 TF/s) and only 12 MB of input DMA + 4 MB out, so the PE is the
bottleneck instead of HBM (the column-sharded variant reads all 32MB of x
on every core).

Schedule: 512 N=512 matmuls in 7 PSUM waves -- one 8-bank wave, then six
4-bank sub-waves alternating bank sets so a wave's banks were freed a full
wave earlier (no start=True stall).  k-outer order matches the k-major DMA
arrival; graduated chunk sizes (128KB singles -> 1MB) cover the ~3.5us DMA
pipeline-fill latency without starving the PE.  ~36 dummy matmuls on
memset scratch pre-warm the PE HAM clock gate during the initial DMA wait
so the real stream runs at 2.4 GHz from the first matmul.
"""

import numpy as np
import ml_dtypes

import concourse.bass as bass  # noqa: F401  (engine types come via bacc)
import concourse.bacc as bacc
import concourse.mybir as mybir
from concourse.tile import TileContext
from concourse.bass_utils import run_bass_kernel_spmd

B, S, D, H, DH = 2, 4096, 2048, 16, 128
N_CORES = 8
P = 128
ROWS = B * S              # 8192
CROWS = ROWS // N_CORES   # 1024 rows per core
KCH = D // P              # 16 contraction chunks over d
RB = 512                  # matmul free dim (PSUM bank limit for f32 out)
NRB = CROWS // RB         # 2 row blocks per core
NT = D // P               # 16 output col tiles of 128
NQ = 4                    # quarter = 4 col tiles = one 4-bank group
N_WARM = 36               # HAM pre-warm dummy matmuls (N=128, ~3.8us cold)

_BF16 = ml_dtypes.bfloat16


def _build_nc():
    f32 = mybir.dt.float32
    bf16 = mybir.dt.bfloat16
    nc = bacc.Bacc(None, target_bir_lowering=False, debug=False)

    # m[p, q*8192 + k*512 + n'] = M[k*128+p, q*512+n']   (q-major so a
    # (q, k-range) chunk is one contiguous per-partition run).
    m = nc.declare_dram_parameter("m", [P, NQ * KCH * RB], bf16, isOutput=False)
    # x[p, rb*8192 + k*512 + r'] = x_core[rb*512+r', k*128+p]
    x = nc.declare_dram_parameter("x", [P, NRB * KCH * RB], bf16, isOutput=False)
    bo = nc.declare_dram_parameter("bo", [P, NT], f32, isOutput=False)
    # out[rb*2048 + nt*128 + p, c] = outT[nt*128+p, rb*512+c]
    out = nc.declare_dram_parameter("out", [NRB * D, RB], bf16, isOutput=True)
    # [p, rb, q, j, c] view so a (rb, q) store is one dma_start
    out_r = out[:].rearrange("(rb q j p) c -> p rb q j c",
                             rb=NRB, q=NQ, j=NQ, p=P)

    MQ = KCH * RB   # 8192: one q block of m's free dim
    XH = KCH * RB   # 8192: one rb block of x's free dim

    def mslice(t, q, k0, k1):
        return t[:, q * MQ + k0 * RB:q * MQ + k1 * RB]

    def xslice(t, rb, k0, k1):
        return t[:, rb * XH + k0 * RB:rb * XH + k1 * RB]

    with TileContext(nc) as tc:
        with (
            tc.tile_pool(name="const", bufs=1) as cp,
            tc.tile_pool(name="ob", bufs=3) as op,
            tc.tile_pool(name="ps", bufs=1, space="PSUM") as pp,
        ):
            m_sb = cp.tile([P, NQ * MQ], bf16)
            x_sb = cp.tile([P, NRB * XH], bf16)
            bo_sb = cp.tile([P, NT], f32)
            warm_a = cp.tile([P, P], bf16)
            nc.vector.memset(warm_a[:], 0.0)
            nc.scalar.dma_start(out=bo_sb[:], in_=bo[:])

            # DMA queue (sync ring, FIFO).  Wave 0 (rb0, nt0-7) consumes
            # x[rb0,k] + m[q0,k] + m[q1,k] per k-step; graduated chunks so
            # the first matmul can start ~4us after the first issue while
            # later chunks hit full DMA efficiency.  Sized so every chunk
            # lands >=0.7us before the PE's k-step needs it (both the
            # ~0.6us per-issue serialization and the ~350GB/s line rate
            # are modeled).
            for k0, k1 in ((0, 1), (1, 2), (2, 3), (3, 5), (5, 8),
                           (8, 12), (12, 16)):
                nc.sync.dma_start(out=xslice(x_sb, 0, k0, k1),
                                  in_=xslice(x, 0, k0, k1))
                nc.sync.dma_start(out=mslice(m_sb, 0, k0, k1),
                                  in_=mslice(m, 0, k0, k1))
                nc.sync.dma_start(out=mslice(m_sb, 1, k0, k1),
                                  in_=mslice(m, 1, k0, k1))
            for q in (2, 3):
                nc.sync.dma_start(out=mslice(m_sb, q, 0, 8),
                                  in_=mslice(m, q, 0, 8))
                nc.sync.dma_start(out=mslice(m_sb, q, 8, 16),
                                  in_=mslice(m, q, 8, 16))
            for k0, k1 in ((0, 8), (8, 16)):
                nc.sync.dma_start(out=xslice(x_sb, 1, k0, k1),
                                  in_=xslice(x, 1, k0, k1))

            # HAM pre-warm: tiny independent matmuls keep the PE busy from
            # ~6us (right after the memsets) so the 4096-cycle activity
            # window un-throttles the clock gate before real data lands.
            warm_ps = pp.tile([P, RB], f32, name="warm", tag="ps7", bufs=1)
            for i in range(N_WARM):
                nc.tensor.matmul(warm_ps[:, 0:P], warm_a[:], warm_a[:],
                                 start=True, stop=True)

            def copy_out(j, ps, obslice, nt):
                if j % 2:
                    nc.scalar.activation(
                        obslice, ps[:],
                        mybir.ActivationFunctionType.Identity,
                        bias=bo_sb[:, nt:nt + 1],
                    )
                else:
                    nc.vector.tensor_scalar_add(
                        obslice, ps[:], bo_sb[:, nt:nt + 1]
                    )

            # Waves: (rb, nt-base, n-banks, bank-base).  Wave 0 spans 8
            # banks (more PE work per k-step while the DMA pipeline fills);
            # then 4-bank groups alternating bank sets {0-3}/{4-7} so
            # start=True never waits on a copy; the final wave is split
            # 2+2 to shorten the copy/store tail after the last matmul.
            waves = [(0, 0, 8, 0)] + [
                (rb, qq * NQ, NQ, (0 if (wi % 2 == 0) else 4))
                for wi, (rb, qq) in enumerate(
                    ((0, 2), (0, 3), (1, 0), (1, 1), (1, 2)))
            ] + [(1, 12, 2, 4), (1, 14, 2, 6)]
            for w, (rb, ntbase, nbanks, bankbase) in enumerate(waves):
                pss = [
                    pp.tile([P, RB], f32, name=f"ps{w}_{j}",
                            tag=f"ps{bankbase + j}", bufs=1)
                    for j in range(nbanks)
                ]
                for k in range(KCH):
                    for j in range(nbanks):
                        nt = ntbase + j
                        q, jq = divmod(nt, NQ)
                        nc.tensor.matmul(
                            pss[j][:],
                            m_sb[:, q * MQ + k * RB + jq * P:
                                 q * MQ + k * RB + (jq + 1) * P],
                            xslice(x_sb, rb, k, k + 1),
                            start=(k == 0),
                            stop=(k == KCH - 1),
                        )
                # copies in j order (so the first banks free earliest for
                # the next wave); one store per bank group on the sync
                # ring, which is idle once the loads are issued.
                for g0 in range(0, nbanks, NQ):
                    gn = min(NQ, nbanks - g0)
                    ob = op.tile([P, gn, RB], bf16, name=f"ob{w}_{g0}",
                                 tag=("ob" if gn == NQ else "ob2"))
                    q, j0q = divmod(ntbase + g0, NQ)
                    for jj in range(gn):
                        nt = ntbase + g0 + jj
                        copy_out(nt, pss[g0 + jj], ob[:, jj, :], nt)
                    nc.sync.dma_start(
                        out=out_r[:, rb, q, j0q:j0q + gn, :],
                        in_=ob[:],
                    )
    nc.compile()
    return nc


_NC = None


def _get_nc():
    global _NC
    if _NC is None:
        _NC = _build_nc()
    return _NC


def prepare_in_maps(normalized_resid_pre, W_V, b_V, W_O, b_O):
    wv2 = np.asarray(W_V, dtype=np.float32).transpose(1, 0, 2).reshape(D, D)
    wo2 = np.asarray(W_O, dtype=np.float32).reshape(D, D)
    bm = wv2 @ wo2  # [d, d'] fp32
    bo_full = (
        np.asarray(b_O, dtype=np.float32)
        + np.asarray(b_V, dtype=np.float32).reshape(D) @ wo2
    )
    # m_host[p, q, k, n'] = M[k*128+p, q*512+n']
    m_host = np.ascontiguousarray(
        bm.astype(_BF16).reshape(KCH, P, NQ, RB).transpose(1, 2, 0, 3)
    ).reshape(P, -1)
    bo_host = np.ascontiguousarray(bo_full.reshape(NT, P).T)  # [P, NT]

    x2 = np.asarray(normalized_resid_pre, dtype=np.float32).reshape(ROWS, D)
    in_maps = []
    for i in range(N_CORES):
        xc = x2[i * CROWS:(i + 1) * CROWS].astype(_BF16)  # [1024, 2048]
        # x_host[p, rb, k, r'] = xc[rb*512+r', k*128+p]
        x_host = np.ascontiguousarray(
            xc.reshape(NRB, RB, KCH, P).transpose(3, 0, 2, 1)
        ).reshape(P, -1)
        in_maps.append({"m": m_host, "x": x_host, "bo": bo_host})
    return in_maps


def assemble_output(results):
    # out[rb, nt, p, c] = outT_core[nt*128+p, rb*512+c]
    outs = [
        np.asarray(r["out"]).reshape(NRB, NT, P, RB).transpose(0, 3, 1, 2)
        .reshape(CROWS, D)
        for r in results
    ]
    full = np.concatenate(outs, axis=0)  # [ROWS, D] bf16
    return np.ascontiguousarray(full.astype(np.float32)).reshape(B, S, D)


def kernel(
    normalized_resid_pre,
    W_Q=None,
    b_Q=None,
    W_K=None,
    b_K=None,
    W_V=None,
    b_V=None,
    W_O=None,
    b_O=None,
    **_unused,
):
    nc = _get_nc()
    in_maps = prepare_in_maps(normalized_resid_pre, W_V, b_V, W_O, b_O)
    last_err = None
    for _attempt in range(3):
        try:
            res = run_bass_kernel_spmd(nc, in_maps, core_ids=list(range(N_CORES)))
            return assemble_output(res.results)
        except Exception as e:  # transient runtime hiccups: retry
            last_err = e
    raise last_err


if __name__ == "__main__":
    rng = np.random.default_rng(0)
    x = rng.standard_normal((B, S, D), dtype=np.float32)
    wq = rng.standard_normal((H, D, DH), dtype=np.float32) * 0.02
    wv = rng.standard_normal((H, D, DH), dtype=np.float32) * 0.02
    wo_ = rng.standard_normal((H, DH, D), dtype=np.float32) * 0.02
    bv = rng.standard_normal((H, DH)).astype(np.float32) * 0.01
    bo_ = rng.standard_normal((D,)).astype(np.float32) * 0.01
    out = kernel(
        x,
        W_Q=wq,
        b_Q=np.zeros((H, DH), np.float32),
        W_K=wq,
        b_K=np.zeros((H, DH), np.float32),
        W_V=wv,
        b_V=bv,
        W_O=wo_,
        b_O=bo_,
    )
    wo2 = wo_.reshape(D, D)
    expect = x.reshape(ROWS, D) @ (
        wv.transpose(1, 0, 2).reshape(D, D) @ wo2
    ) + (bo_ + bv.reshape(D) @ wo2)
    expect = expect.reshape(B, S, D)
    err = np.abs(out - expect).max() / np.abs(expect).max()
    print("quick self-check rel abs err:", err)
